# revision 36
# baseline (speedup 1.0000x reference)
# Block-circulant linear kernel for Trainium2 (Bass/Tile), 8-core SPMD.
#
# y[b, 16m+p] = sum_{n,q} blocks[(m-n)%512, p, q] * x[b, 16n+q]
#
# Strategy: shard the output block axis m across 8 cores (64 block-rows each).
# Per core, store a doubled+shifted "BIGQ" layout of blocks in SBUF:
#     BIGQ[(ni,q), u*16+p] = blocks[(m0 + u - ni) % 512, p, q]
# so that EVERY 128x128 weight tile of the implied 8192x8192 circulant matrix
# is a contiguous 128-column slice of BIGQ (the circulant gather becomes pure
# addressing). All (m_tile t, n_chunk c) pairs with the same diagonal offset
# d = t - c share one stationary tile, so the whole per-core compute is 71
# accumulating matmuls into a single PSUM bank [128 mp, 8 t x 32 b].
#
# The xt layout is reversed (c' = 63 - c) and the psum t axis flipped
# (t' = 7 - t) so both the weight stream (BIGQ u ascending) and the moving
# stream (xt c' ascending) are consumed in DMA arrival order.
#
# vs the 28.6us tile-based baseline (now ~24.3-25.2us):
#  - raw bass, no TileContext: the tile stage preamble delayed the first DMA
#    ~0.9us and the stage close added ~1.4us of per-DMA-semaphore drains.
#    Manual semaphores give exact dependency edges instead.
#  - all input DMAs on ONE HWDGE ring (sync) in strict consumption order:
#    the two rings round-robin at packet granularity, so a second ring's
#    later chunks steal SDMA bandwidth from the next-needed chunk and spread
#    its completion increments over ~2.4us. Single-ring FIFO makes
#    completion order = consumption order (and is far less run-variable).
#  - fp16 dummy matmuls (on resident garbage) interleaved in front of each
#    chunk's first consumer, sized from a calibrated supply-curve model:
#    any PE idle >~0.3us resets the HAM activity window and drops the PE
#    clock 2.4->1.2GHz for 3.4us+, so stalls are converted into clock-
#    keeping work. BIGQ cols [0,128) are never loaded (u0 starts at 8).
#  - output copy+DMA strictly after the last matmul: a DVE/ACT PSUM read
#    concurrent with PE writes to the same bank crashes the device (found
#    by bisection; ACT .copy from PSUM crashes even post-stop).
import os
import numpy as np

B = 32
NB = 512          # number of 16x16 blocks
NCORES = 8
MBLK = NB // NCORES   # 64 output block-rows per core
W = 576               # BIGQ window width (in u units of 16 columns)
ND = 71               # diagonal offsets d in [-63, 7]

DTYPE = "float16"
NWARM = int(os.environ.get("KNWARM", "2"))
SPLIT_OUT = os.environ.get("KSPLIT_OUT", "1") == "1"
PAD_EXTRA = float(os.environ.get("KPAD", "0.12"))  # extra us of dummy pad per chunk

_cached = {}
_last_results = None  # BassKernelResults of the most recent run (for profiling)


def _np_dtype(name):
    if name == "bfloat16":
        import ml_dtypes

        return ml_dtypes.bfloat16
    if name == "float16":
        return np.float16
    return np.float32


def _split_dt(dt_name):
    """'wt:mv' -> (weight dtype, moving dtype); single name -> same both."""
    if ":" in dt_name:
        wt, mv = dt_name.split(":")
        return wt, mv
    return dt_name, dt_name


def _build_program(dt_name):
    import concourse.bacc as bacc
    import concourse.mybir as mybir

    wt_name, mv_name = _split_dt(dt_name)
    wdt = getattr(mybir.dt, wt_name)
    mdt = getattr(mybir.dt, mv_name)
    f32 = mybir.dt.float32

    # Bacc (not plain Bass): its compile() pipeline splits multi-wait
    # instructions into EventSemaphore preludes (HW allows 1 wait/inst).
    # Raw bass (no TileContext): the tile stage preamble costs ~0.9us before
    # the first DMA can issue and the stage close another ~1.4us of per-sem
    # drains; manual semaphores avoid both and give exact dependency edges
    # (tile tracked the psum accumulator at whole-tile granularity, forcing
    # the first output copy to wait for the very last matmul).
    nc = bacc.Bacc("TRN2", target_bir_lowering=False, debug=False, num_devices=NCORES)
    xt_d = nc.declare_dram_parameter("xt", [128, 2048], mdt, isOutput=False)
    bq_d = nc.declare_dram_parameter("bigq", [128, W * 16], wdt, isOutput=False)
    out_d = nc.declare_dram_parameter("out", [128, 256], f32, isOutput=True)

    if True:
        if True:
            xt = nc.alloc_sbuf_tensor("xt_sb", [128, 2048], mdt)
            bq = nc.alloc_sbuf_tensor("bq_sb", [128, W * 16], wdt)
            out_sb = nc.alloc_sbuf_tensor("out_sb", [128, 256], f32)
            warm_sb = nc.alloc_sbuf_tensor("warm_sb", [128, 256], mdt)
            # full-bank allocations so the real accumulation group (acc) and
            # the interleaved dummy group (warm_ps) live in separate PSUM
            # banks; a start=True clear only affects its own bank.
            acc = nc.alloc_psum_tensor("acc", [128, 512], f32)
            warm_ps = nc.alloc_psum_tensor("warm_ps", [128, 512], f32)

            # Input DMA schedule, in matmul consumption order. BIGQ cols
            # [0,128) are never read (stationary tiles start at u0=8), so the
            # bigq stream covers [128, 9216) in fine ~256KB chunks: each
            # chunk's completion semaphore fires ~1us after its last byte
            # (HBM receipt round-trip), and the PE can only consume whole
            # chunks, so finer chunks keep the PE fed at lower latency.
            # (first_diag, tensor, dram, lo, hi): first_diag = first matmul
            # diagonal that reads this chunk (for the pacing model below).
            order = [
                (0, xt, xt_d, 0, 512),        # xt cols for diagonals <= 15
                (0, bq, bq_d, 128, 640),      # diagonals 0-3
                (4, bq, bq_d, 640, 1664),     # diagonals 4-11
                (16, xt, xt_d, 512, 1280),    # xt cols, enough through i<=39
                (12, bq, bq_d, 1664, 2688),   # diagonals 12-19
                (20, bq, bq_d, 2688, 3712),   # diagonals 20-27
                (40, xt, xt_d, 1280, 2048),   # xt rest (needed from diag 40)
                (28, bq, bq_d, 3712, 4736),   # diagonals 28-35
                (36, bq, bq_d, 4736, 5760),   # diagonals 36-43
                (44, bq, bq_d, 5760, 6784),   # diagonals 44-51
                (52, bq, bq_d, 6784, 7808),   # diagonals 52-59
                (60, bq, bq_d, 7808, 8832),   # diagonals 60-67
                (68, bq, bq_d, 8832, 9216),   # diagonals 68-70
            ]
            # rings partitioned by tensor: bq (2.2MB) on sync, xt (0.5MB)
            # on scalar. Each ring's FIFO = its own consumption order, so
            # the packet round-robin between rings only overlaps streams
            # that are both needed early; bq's mid/late chunk semaphores no
            # longer queue behind xt bytes.
            dma_sems = []
            for k, (_, tile_, dram_, lo, hi) in enumerate(order):
                s = nc.alloc_semaphore(f"dma{k}")
                e = nc.scalar if tile_ is xt else nc.sync
                e.dma_start(
                    tile_.ap()[:, lo:hi], dram_.ap()[:, lo:hi]
                ).then_inc(s, 16)
                dma_sems.append(s)

            # --- PE pacing model ------------------------------------------
            # Supply curve:
            #   issue_end(k) = T_DMA0 + 0.66*(k+1)  (per-DMA descriptor gen)
            #   data_end(k)  = max(issue_end(k), first_byte + cumKB(k)/RATE)
            #   sem(k)       = data_end(k) + SEM_LAG  (HBM receipt round-trip)
            # RATE is calibrated conservatively (~310KB/us; bursts reach 390
            # but the per-NC HBM share drifts run to run) — over-padding
            # costs ~55ns per dummy, under-padding costs a 1.2GHz re-throttle.
            # The PE must never sit idle on a semaphore wait: any >~0.3us
            # idle resets the HAM activity window and drops the PE clock to
            # 1.2GHz for 3.4us+ (measured: one 0.6us stall cost ~2.2us).
            # So we interleave fp16 dummy matmuls (N=128 on resident garbage)
            # in front of the first consumer of each chunk, sized to absorb
            # the predicted wait. They cost ~55ns each when supply is ahead
            # but keep the clock warm when it isn't.
            T_DMA0 = 6.5       # first dma_start issue (no tile stage barrier)
            T_PE0 = 6.6        # tensor engine ready (after memset of warm_sb)
            FIRST_BYTE = 7.6
            RATE = float(os.environ.get("KRATE", "310"))  # KB/us supply
            SEM_LAG = float(os.environ.get("KLAG", "1.0"))
            COLD, WARMT = 1.2, 2.4

            cum = 0.0
            sem_t = {}
            for k, (fd, tile_, dram_, lo, hi) in enumerate(order):
                cum += (hi - lo) * 256 / 1024.0   # KB (2-byte dtypes, 128 rows)
                issue_end = T_DMA0 + 0.66 * (k + 1)
                sem_t[k] = max(issue_end, FIRST_BYTE + cum / RATE) + SEM_LAG
            # first_diag -> (sem time, [chunk indices to wait on])
            chunk_by_diag = {}
            for k, (fd, *_rest) in enumerate(order):
                ent = chunk_by_diag.setdefault(fd, [0.0, []])
                ent[0] = max(ent[0], sem_t[k])
                ent[1].append(k)

            ms_sem = nc.alloc_semaphore("ms")
            nc.vector.memset(warm_sb.ap(), 0.0).then_inc(ms_sem, 1)
            nc.tensor.wait_ge(ms_sem, 1)

            hi_done = nc.alloc_semaphore("hi_done")
            lo_done = nc.alloc_semaphore("lo_done")

            warm_flip = T_PE0 + 3.4   # HAM flips to 2.4GHz here if no idle
            t = T_PE0
            ndummy = 0

            def emit_dummies(until):
                nonlocal t, ndummy
                while t < until and os.environ.get("KNODUMMY") != "1":
                    freq = WARMT if t >= warm_flip else COLD
                    dur = (128.0 / freq + 4.0) / 1000.0
                    nc.tensor.matmul(
                        warm_ps.ap()[:, 0:128],
                        warm_sb.ap()[:, 0:128], warm_sb.ap()[:, 0:128],
                        start=(ndummy == 0), stop=False, skip_group_check=True,
                    )
                    ndummy += 1
                    t += dur

            ND_RUN = int(os.environ.get("KNDRUN", str(ND)))
            # d = t - c diagonal; stationary tile = BIGQ columns [16*u0, 16*u0+128)
            # with u0 = 8*i + 8 for i = 0..70 (d = i - 63).
            for i in range(ND_RUN):
                d = i - 63
                u0 = 8 * i + 8
                t_lo = max(0, d)
                t_hi = min(7, 63 + d)
                nt = t_hi - t_lo + 1
                tp_lo = 7 - t_hi           # flipped psum tile index
                cp_lo = 63 + d - t_hi      # reversed xt chunk index
                if i in chunk_by_diag:
                    sem_time, ks = chunk_by_diag[i]
                    emit_dummies(sem_time + PAD_EXTRA)
                    for k in ks:
                        nc.tensor.wait_ge(dma_sems[k], 16)
                mm = nc.tensor.matmul(
                    acc.ap()[:, 32 * tp_lo: 32 * (tp_lo + nt)],
                    bq.ap()[:, 16 * u0: 16 * u0 + 128],
                    xt.ap()[:, 32 * cp_lo: 32 * (cp_lo + nt)],
                    start=(i == 0),   # clears the whole PSUM bank
                    stop=(i == ND_RUN - 1),
                    skip_group_check=True,
                )
                if i == ND_RUN - 1:
                    mm.then_inc(lo_done, 2)
                freq = WARMT if t >= warm_flip else COLD
                t += (nt * 32.0 / freq + 4.0) / 1000.0
            # close the dummy accumulation group
            nc.tensor.matmul(
                warm_ps.ap()[:, 0:128],
                warm_sb.ap()[:, 0:128], warm_sb.ap()[:, 0:128],
                start=False, stop=True, skip_group_check=True,
            )

            # Output path, entirely after the last matmul (a PSUM read
            # concurrent with PE writes to the same bank crashes the device;
            # found by bisection). The two 64KB halves run on disjoint
            # engine/ring pairs: vector copy -> sync DMA, scalar copy ->
            # scalar DMA, so the copy+issue+receipt chains overlap.
            cp_hi = nc.alloc_semaphore("cp_hi")
            cp_lo = nc.alloc_semaphore("cp_lo")
            od_hi = nc.alloc_semaphore("od_hi")
            od_lo = nc.alloc_semaphore("od_lo")
            nc.vector.wait_ge(lo_done, 2)
            nc.vector.tensor_copy(
                out_sb.ap()[:, 0:256], acc.ap()[:, 0:256]
            ).then_inc(cp_lo, 1)
            nc.sync.wait_ge(cp_lo, 1)
            nc.sync.dma_start(
                out_d.ap()[:, 0:256], out_sb.ap()[:, 0:256]
            ).then_inc(od_lo, 16)
            # hold the program open until the output write is confirmed
            nc.sync.wait_ge(od_lo, 16)
            nc.sync.drain()
    nc.compile()
    return nc


def _get_program(dt_name):
    key = (dt_name, NWARM, SPLIT_OUT, PAD_EXTRA)
    if key not in _cached:
        _cached[key] = _build_program(dt_name)
    return _cached[key]


def _prep_inputs(x, blocks, dt_name):
    """Host-side layout prep (pure numpy reshuffles of the small inputs)."""
    x = np.ascontiguousarray(np.asarray(x), dtype=np.float32)
    blocks = np.ascontiguousarray(np.asarray(blocks), dtype=np.float32)
    # xt[(ni*16+q), c*32+b] = x[b, 128c + 16ni + q], then reverse c (c'=63-c)
    xt = x.T.reshape(64, 128, 32).transpose(1, 0, 2)[:, ::-1, :].reshape(128, 2048)
    xt = np.ascontiguousarray(xt)
    u = np.arange(W)
    ni = np.arange(8)
    wt_name, mv_name = _split_dt(dt_name)
    np_w, np_m = _np_dtype(wt_name), _np_dtype(mv_name)
    xt_c = np.ascontiguousarray(xt.astype(np_m))
    in_maps = []
    for k in range(NCORES):
        m0 = k * MBLK
        idx = (m0 + u[None, :] - ni[:, None]) % NB        # [8, W]
        bigq = blocks[idx]                                 # [8, W, p, q]
        bigq = bigq.transpose(0, 3, 1, 2).reshape(128, W * 16)  # [(ni,q), (u,p)]
        in_maps.append(
            {"xt": xt_c, "bigq": np.ascontiguousarray(bigq.astype(np_w))}
        )
    return in_maps


def _assemble(results):
    y = np.empty((B, NB * 16), dtype=np.float32)
    for k in range(NCORES):
        o = np.asarray(results[k]["out"])  # [128 (mi,p), 256 (t',b)], t = 7-t'
        y[:, 1024 * k: 1024 * (k + 1)] = (
            o.reshape(128, 8, 32)[:, ::-1, :].transpose(2, 1, 0).reshape(32, 1024)
        )
    return y


def kernel(x, blocks):
    global _last_results
    from concourse.bass_utils import run_bass_kernel_spmd

    nc = _get_program(DTYPE)
    in_maps = _prep_inputs(x, blocks, DTYPE)
    res = run_bass_kernel_spmd(nc, in_maps, list(range(NCORES)))
    _last_results = res
    return _assemble(res.results)
